# revision 6
# baseline (speedup 1.0000x reference)
"""Multi-head attention (B=2, S=2048, E=1024, H=16, D=64) on 8 Trainium2 cores.

Sharding: data-parallel over batch (2 groups of 4 cores), tensor-parallel over
heads within each group (4 heads per core, Megatron-style column-split qkv);
out_proj is sharded over its OUTPUT columns - each core receives its own
w_out[:, r*256:(r+1)*256] slice from the host, so the SPMD program contains
no core-id logic at all.

v2 design (all PE inputs bf16, fp32 PSUM accumulation):
  - x arrives pre-transposed from the host (x^T [1024, 2048] bf16), so the
    kernel spends zero PE cycles on transposes.
  - Projections are emitted span-wise (4 x 512 seq columns) so the matmuls
    stream behind the x^T DMA: kT for all spans first, then qT(span0), then
    v (needed progressively by the PV loop), then qT(1..3).
  - Attention per 512-row q-chunk: per k-chunk, two head-pair chains: S^T
    with two heads row-packed as K=64 pairs (concurrent on the PE), exp on
    ScalarE straight out of PSUM (scale=1/8 folded in; logits ~N(0,1) so no
    max subtraction needed), fused PV^T + softmax denominator via
    lhsT=[v_h | ones] accumulated over the 16 k-chunks.
  - PSUM: ONE shared 2-buf [128,1024] pool ("mm") serves qkv-projection
    tiles, S^T tiles, and out_proj tiles (4 banks), plus a 2-buf PV
    accumulator pool (4 banks) = exactly 8 banks, so projections and
    out_proj overlap attention with no pool barrier.
  - One AllGather per body ([256,2048] bf16 = 1MB/rank; more collectives
    per NEFF crash the current axon worker): in steady state (reps>1 or
    back-to-back calls) the gather + out_proj pipeline under the NEXT
    body's projection phase, so they are off the critical path; out_proj
    PSUM comes from the PV pool, which is idle during projections.
"""

import numpy as np
from contextlib import ExitStack

import concourse.tile as tile
from concourse import bacc, mybir
from concourse.bass_utils import run_bass_kernel_spmd

B, S, E, H, D = 2, 2048, 1024, 16, 64
N_CORES = 8
HPC = 4            # heads per core
HD = HPC * D       # 256
GROUPS = [[0, 1, 2, 3], [4, 5, 6, 7]]

F32 = mybir.dt.float32
BF16 = mybir.dt.bfloat16

_cached = None


def build(reps=1, profile=False):
    nc = bacc.Bacc("TRN2", target_bir_lowering=False, debug=False,
                   num_devices=N_CORES)

    xT_d = nc.dram_tensor("xT", [E, S], BF16, kind="ExternalInput").ap()
    wq_d = nc.dram_tensor("wq", [E, HD], BF16, kind="ExternalInput").ap()
    wk_d = nc.dram_tensor("wk", [E, HD], BF16, kind="ExternalInput").ap()
    wv_d = nc.dram_tensor("wv", [E, HD], BF16, kind="ExternalInput").ap()
    wo_d = nc.dram_tensor("wo", [E, HD], BF16, kind="ExternalInput").ap()
    y_d = nc.dram_tensor("y", [S, HD], F32, kind="ExternalOutput").ap()
    cc_in = nc.dram_tensor("cc_in", [2 * 128, S], BF16).ap()
    cc_out = nc.dram_tensor("cc_out", [8 * 128, S], BF16).ap()

    with tile.TileContext(nc) as tc, ExitStack() as ctx:
        glob = ctx.enter_context(tc.tile_pool(name="glob", bufs=1))
        xT_t = glob.tile([128, 8, S], BF16, tag="xT")
        wq_t = glob.tile([128, 8, HD], BF16, tag="wq")
        wk_t = glob.tile([128, 8, HD], BF16, tag="wk")
        wv_t = glob.tile([128, 8, HD], BF16, tag="wv")
        wo_t = glob.tile([128, 8, HD], BF16, tag="wo")
        qT_t = glob.tile([128, 2, S], BF16, tag="qT")     # q^T by head pair
        kT_t = glob.tile([128, 2, S], BF16, tag="kT")
        v_t = glob.tile([128, 16, HPC, 128], BF16, tag="v")  # [v_h | ones]
        ones_b = glob.tile([128, D], BF16, tag="ones_b")

        nc.sync.dma_start(wq_t[:], wq_d.rearrange("(c p) n -> p c n", p=128))
        nc.sync.dma_start(wk_t[:], wk_d.rearrange("(c p) n -> p c n", p=128))
        nc.sync.dma_start(wv_t[:], wv_d.rearrange("(c p) n -> p c n", p=128))
        nc.sync.dma_start(wo_t[:], wo_d.rearrange("(c p) n -> p c n", p=128))
        nc.gpsimd.memset(ones_b[:], 1.0)
        # ones half of every v_aug block (written once; v halves re-written
        # by the projection each rep)
        for sc in range(16):
            for h in range(HPC):
                nc.vector.tensor_copy(v_t[:, sc, h, 64:128], ones_b[:])

        for _rep in range(reps):
            _emit_body(nc, tc, xT_d, y_d, cc_in, cc_out,
                       xT_t, wq_t, wk_t, wv_t, wo_t, qT_t, kT_t, v_t,
                       profile)

    nc.compile()
    return nc


def _emit_body(nc, tc, xT_d, y_d, cc_in, cc_out,
               xT_t, wq_t, wk_t, wv_t, wo_t, qT_t, kT_t, v_t, profile):
    xT_r = xT_d.rearrange("(c p) s -> p c s", p=128)

    with ExitStack() as body:
        mm = body.enter_context(tc.tile_pool(name="mm", bufs=2, space="PSUM"))
        fzp = body.enter_context(tc.tile_pool(name="fzp", bufs=2,
                                              space="PSUM"))
        expp = body.enter_context(tc.tile_pool(name="expp", bufs=4))
        recp = body.enter_context(tc.tile_pool(name="recp", bufs=2))
        outp = body.enter_context(tc.tile_pool(name="outp", bufs=2))
        otp = body.enter_context(tc.tile_pool(name="otp", bufs=2))
        ysb = body.enter_context(tc.tile_pool(name="ysb", bufs=3))

        # ---- x^T spans stream in; kT per span as soon as its span lands ----
        for z in range(4):
            nc.sync.dma_start(xT_t[:, :, z * 512:(z + 1) * 512],
                              xT_r[:, :, z * 512:(z + 1) * 512])

        def proj_T(w_t, dst, z):
            # dst[:, mc, span] = (w col-block mc)^T @ x^T span
            for mc in range(2):
                pp = mm.tile([128, 1024], F32, tag="st")
                for ec in range(8):
                    nc.tensor.matmul(
                        pp[:, 0:512],
                        w_t[:, ec, mc * 128:(mc + 1) * 128],
                        xT_t[:, ec, z * 512:(z + 1) * 512],
                        start=(ec == 0), stop=(ec == 7))
                nc.vector.tensor_copy(
                    dst[:, mc, z * 512:(z + 1) * 512], pp[:, 0:512])

        def proj_v(z):
            for i in range(4):
                sc = z * 4 + i
                pp = mm.tile([128, 1024], F32, tag="st")
                for ec in range(8):
                    nc.tensor.matmul(
                        pp[:, 0:HD],
                        xT_t[:, ec, sc * 128:(sc + 1) * 128],
                        wv_t[:, ec, :],
                        start=(ec == 0), stop=(ec == 7))
                nc.vector.tensor_copy(
                    v_t[:, sc, :, 0:64],
                    pp[:, 0:HD].rearrange("p (h d) -> p h d", h=HPC))

        for z in range(4):
            proj_T(wk_t, kT_t, z)
        proj_T(wq_t, qT_t, 0)
        for z in range(4):
            proj_v(z)
        for z in range(1, 4):
            proj_T(wq_t, qT_t, z)

        # ---- attention + chunked gather + out_proj, per 512-row q-chunk ----
        for qc in range(4):
            fz0 = fzp.tile([128, 2, 512], F32, tag="pv")
            fz1 = fzp.tile([128, 2, 512], F32, tag="pv")
            fzs = [fz0, fz1]
            for kc in range(16):
                sts = []
                for hp in range(2):
                    st = mm.tile([128, 1024], F32, tag="st")
                    for par in range(2):   # row-packed K=64 pair
                        lo, hi = par * 64, (par + 1) * 64
                        nc.tensor.matmul(
                            st[:, par * 512:(par + 1) * 512],
                            kT_t[lo:hi, hp, kc * 128:(kc + 1) * 128],
                            qT_t[lo:hi, hp, qc * 512:(qc + 1) * 512],
                            start=True, stop=True)
                    sts.append(st)
                exs = []
                for hp in range(2):
                    ex = expp.tile([128, 1024], BF16, tag="ex")
                    nc.scalar.activation(
                        ex[:], sts[hp][:],
                        mybir.ActivationFunctionType.Exp, scale=0.125)
                    exs.append(ex)
                for hp in range(2):
                    for par in range(2):
                        h = 2 * hp + par
                        nc.tensor.matmul(
                            fzs[hp][:, par, :],
                            v_t[:, kc, h, :],
                            exs[hp][:, par * 512:(par + 1) * 512],
                            start=(kc == 0), stop=(kc == 15))
            outT = outp.tile([128, 2, 512], BF16, tag="outT")
            for hp in range(2):
                for par in range(2):
                    rc = recp.tile([64, 512], F32, tag="rc")
                    nc.vector.reciprocal(rc[:], fzs[hp][64:128, par, :])
                    nc.vector.tensor_mul(
                        outT[par * 64:(par + 1) * 64, hp, :],
                        fzs[hp][0:64, par, :], rc[:])
            for hp in range(2):
                nc.sync.dma_start(
                    cc_in[hp * 128:(hp + 1) * 128,
                          qc * 512:(qc + 1) * 512],
                    outT[:, hp, :])

        # one gather per body; off the critical path once reps pipeline
        if profile:
            # single-core stand-in with the same data volume as the gather
            for r in range(4):
                nc.sync.dma_start(cc_out[r * 256:(r + 1) * 256, :], cc_in[:])
        else:
            nc.gpsimd.collective_compute(
                "AllGather", mybir.AluOpType.bypass, replica_groups=GROUPS,
                ins=[cc_in[:]], outs=[cc_out[:]])

        for sq in range(4):
            ot = otp.tile([128, 8, 512], BF16, tag="ot")
            nc.sync.dma_start(
                ot[:], cc_out.rearrange("(c p) s -> p c s", p=128)
                [:, :, sq * 512:(sq + 1) * 512])
            for mc in range(4):
                # fzp (not mm): during the NEXT body's projection phase the
                # fz bufs are idle, so gather+out_proj pipeline under it;
                # putting these on mm would make the next projections queue
                # behind the AllGather in the mm rotation.
                ep = fzp.tile([128, 2, 512], F32, tag="pv")
                for hc in range(8):
                    nc.tensor.matmul(
                        ep[:, 0, 0:HD],
                        ot[:, hc, mc * 128:(mc + 1) * 128],
                        wo_t[:, hc, :],
                        start=(hc == 0), stop=(hc == 7))
                yt = ysb.tile([128, HD], F32, tag="y")
                nc.vector.tensor_copy(yt[:], ep[:, 0, 0:HD])
                nc.sync.dma_start(
                    y_d[(sq * 4 + mc) * 128:(sq * 4 + mc + 1) * 128, :],
                    yt[:])


def _get_nc():
    global _cached
    if _cached is None:
        _cached = build()
    return _cached


def _bf16(a):
    import ml_dtypes
    return np.ascontiguousarray(a, dtype=ml_dtypes.bfloat16)


def make_in_maps(x, w_qkv, w_out):
    x = np.asarray(x, dtype=np.float32)
    w_qkv = np.asarray(w_qkv, dtype=np.float32)
    w_out = np.asarray(w_out, dtype=np.float32)
    in_maps = []
    for c in range(N_CORES):
        b, r = c // 4, c % 4
        hs = r * HD                      # first qkv column of this core's heads
        in_maps.append({
            "xT": _bf16(x[b].T),
            "wq": _bf16(w_qkv[:, hs:hs + HD]),
            "wk": _bf16(w_qkv[:, E + hs:E + hs + HD]),
            "wv": _bf16(w_qkv[:, 2 * E + hs:2 * E + hs + HD]),
            "wo": _bf16(w_out[:, r * HD:(r + 1) * HD]),
        })
    return in_maps


def assemble(results):
    y = np.empty((B, S, E), dtype=np.float32)
    for c in range(N_CORES):
        b, r = c // 4, c % 4
        y[b, :, r * HD:(r + 1) * HD] = results[c]["y"]
    return y


def kernel(x, w_qkv, w_out):
    nc = _get_nc()
    res = run_bass_kernel_spmd(nc, make_in_maps(x, w_qkv, w_out),
                               list(range(N_CORES)))
    return assemble(res.results)


# revision 8
# speedup vs baseline: 2.4520x; 2.4520x over previous
"""Multi-head attention (B=2, S=2048, E=1024, H=16, D=64) on 8 Trainium2 cores.

Sharding: data-parallel over batch (2 groups of 4 cores), tensor-parallel over
heads within each group (4 heads per core, Megatron-style column-split qkv).
out_proj is sharded over its INPUT rows: each core multiplies its own 256
attention-output rows (4 heads x 64) by w_out[r*256:(r+1)*256, :] and emits a
full-width partial product y_r [2048, 1024]; the host sums the 4 partials per
batch during unsharding. That removes the device collective entirely (the
on-device AllGather measured ~0.5 ms serialized on this fabric, dominating
the kernel), at the cost of a 4x larger (8 MB) output DMA that overlaps
compute. The SPMD program contains no core-id logic at all.

All PE inputs bf16, fp32 PSUM accumulation:
  - x arrives pre-transposed from the host (x^T [1024, 2048] bf16), so the
    kernel spends zero PE cycles on transposes.
  - Projections are emitted span-wise (4 x 512 seq columns) so the matmuls
    stream behind the x^T DMA: kT for all spans first, then qT(span0), then
    v (needed progressively by the PV loop), then qT(1..3).
  - Attention per 512-row q-chunk: per k-chunk, two head-pair chains: S^T
    with two heads row-packed as K=64 pairs (concurrent on the PE), exp on
    ScalarE straight out of PSUM (scale=1/8 folded in; logits ~N(0,1) so no
    max subtraction needed), fused PV^T + softmax denominator via
    lhsT=[v_h | ones] accumulated over the 16 k-chunks.
  - PSUM: ONE shared 2-buf [128,1024] pool ("mm") serves qkv-projection
    tiles and S^T tiles (4 banks), plus a 2-buf pool (4 banks) shared by the
    PV accumulators and the out_proj partials = exactly 8 banks, so
    projections and out_proj overlap attention with no pool barrier.
  - out_proj(qc) depends only on this core's outT(qc), so it pipelines
    right behind each q-chunk's normalization.
"""

import numpy as np
from contextlib import ExitStack

import concourse.tile as tile
from concourse import bacc, mybir
from concourse.bass_utils import run_bass_kernel_spmd

B, S, E, H, D = 2, 2048, 1024, 16, 64
N_CORES = 8
HPC = 4            # heads per core
HD = HPC * D       # 256

F32 = mybir.dt.float32
BF16 = mybir.dt.bfloat16

_cached = None


def build(reps=1, profile=False):
    nc = bacc.Bacc("TRN2", target_bir_lowering=False, debug=False,
                   num_devices=N_CORES)

    xT_d = nc.dram_tensor("xT", [E, S], BF16, kind="ExternalInput").ap()
    wq_d = nc.dram_tensor("wq", [E, HD], BF16, kind="ExternalInput").ap()
    wk_d = nc.dram_tensor("wk", [E, HD], BF16, kind="ExternalInput").ap()
    wv_d = nc.dram_tensor("wv", [E, HD], BF16, kind="ExternalInput").ap()
    wo_d = nc.dram_tensor("wo", [HD, E], BF16, kind="ExternalInput").ap()
    y_d = nc.dram_tensor("y", [S, E], F32, kind="ExternalOutput").ap()

    with tile.TileContext(nc) as tc, ExitStack() as ctx:
        glob = ctx.enter_context(tc.tile_pool(name="glob", bufs=1))
        xT_t = glob.tile([128, 8, S], BF16, tag="xT")
        wq_t = glob.tile([128, 8, HD], BF16, tag="wq")
        wk_t = glob.tile([128, 8, HD], BF16, tag="wk")
        wv_t = glob.tile([128, 8, HD], BF16, tag="wv")
        wo_t = glob.tile([128, 2, E], BF16, tag="wo")
        qT_t = glob.tile([128, 2, S], BF16, tag="qT")     # q^T by head pair
        kT_t = glob.tile([128, 2, S], BF16, tag="kT")
        v_t = glob.tile([128, 16, HPC, 128], BF16, tag="v")  # [v_h | ones]
        ones_b = glob.tile([128, D], BF16, tag="ones_b")

        nc.sync.dma_start(wq_t[:], wq_d.rearrange("(c p) n -> p c n", p=128))
        nc.sync.dma_start(wk_t[:], wk_d.rearrange("(c p) n -> p c n", p=128))
        nc.sync.dma_start(wv_t[:], wv_d.rearrange("(c p) n -> p c n", p=128))
        nc.sync.dma_start(wo_t[:], wo_d.rearrange("(c p) n -> p c n", p=128))
        nc.gpsimd.memset(ones_b[:], 1.0)
        # ones half of every v_aug block (written once; v halves re-written
        # by the projection each rep)
        for sc in range(16):
            for h in range(HPC):
                nc.vector.tensor_copy(v_t[:, sc, h, 64:128], ones_b[:])

        for _rep in range(reps):
            _emit_body(nc, tc, xT_d, y_d,
                       xT_t, wq_t, wk_t, wv_t, wo_t, qT_t, kT_t, v_t)

    nc.compile()
    return nc


def _emit_body(nc, tc, xT_d, y_d,
               xT_t, wq_t, wk_t, wv_t, wo_t, qT_t, kT_t, v_t):
    xT_r = xT_d.rearrange("(c p) s -> p c s", p=128)

    with ExitStack() as body:
        mm = body.enter_context(tc.tile_pool(name="mm", bufs=2, space="PSUM"))
        fzp = body.enter_context(tc.tile_pool(name="fzp", bufs=2,
                                              space="PSUM"))
        expp = body.enter_context(tc.tile_pool(name="expp", bufs=4))
        recp = body.enter_context(tc.tile_pool(name="recp", bufs=2))
        outp = body.enter_context(tc.tile_pool(name="outp", bufs=2))
        ysb = body.enter_context(tc.tile_pool(name="ysb", bufs=3))

        # ---- x^T spans stream in; kT per span as soon as its span lands ----
        for z in range(4):
            nc.sync.dma_start(xT_t[:, :, z * 512:(z + 1) * 512],
                              xT_r[:, :, z * 512:(z + 1) * 512])

        def proj_T(w_t, dst, z):
            # dst[:, mc, span] = (w col-block mc)^T @ x^T span
            for mc in range(2):
                pp = mm.tile([128, 1024], F32, tag="st")
                for ec in range(8):
                    nc.tensor.matmul(
                        pp[:, 0:512],
                        w_t[:, ec, mc * 128:(mc + 1) * 128],
                        xT_t[:, ec, z * 512:(z + 1) * 512],
                        start=(ec == 0), stop=(ec == 7))
                nc.vector.tensor_copy(
                    dst[:, mc, z * 512:(z + 1) * 512], pp[:, 0:512])

        def proj_v(z):
            for i in range(4):
                sc = z * 4 + i
                pp = mm.tile([128, 1024], F32, tag="st")
                for ec in range(8):
                    nc.tensor.matmul(
                        pp[:, 0:HD],
                        xT_t[:, ec, sc * 128:(sc + 1) * 128],
                        wv_t[:, ec, :],
                        start=(ec == 0), stop=(ec == 7))
                nc.vector.tensor_copy(
                    v_t[:, sc, :, 0:64],
                    pp[:, 0:HD].rearrange("p (h d) -> p h d", h=HPC))

        for z in range(4):
            proj_T(wk_t, kT_t, z)
        proj_T(wq_t, qT_t, 0)
        for z in range(4):
            proj_v(z)
        for z in range(1, 4):
            proj_T(wq_t, qT_t, z)

        # ---- attention + row-partial out_proj, per 512-row q-chunk ----
        for qc in range(4):
            fz0 = fzp.tile([128, 2, 512], F32, tag="pv")
            fz1 = fzp.tile([128, 2, 512], F32, tag="pv")
            fzs = [fz0, fz1]
            for kc in range(16):
                sts = []
                for hp in range(2):
                    st = mm.tile([128, 1024], F32, tag="st")
                    for par in range(2):   # row-packed K=64 pair
                        lo, hi = par * 64, (par + 1) * 64
                        nc.tensor.matmul(
                            st[:, par * 512:(par + 1) * 512],
                            kT_t[lo:hi, hp, kc * 128:(kc + 1) * 128],
                            qT_t[lo:hi, hp, qc * 512:(qc + 1) * 512],
                            start=True, stop=True)
                    sts.append(st)
                exs = []
                for hp in range(2):
                    ex = expp.tile([128, 1024], BF16, tag="ex")
                    nc.scalar.activation(
                        ex[:], sts[hp][:],
                        mybir.ActivationFunctionType.Exp, scale=0.125)
                    exs.append(ex)
                for hp in range(2):
                    for par in range(2):
                        h = 2 * hp + par
                        nc.tensor.matmul(
                            fzs[hp][:, par, :],
                            v_t[:, kc, h, :],
                            exs[hp][:, par * 512:(par + 1) * 512],
                            start=(kc == 0), stop=(kc == 15))
            outT = outp.tile([128, 2, 512], BF16, tag="outT")
            for hp in range(2):
                for par in range(2):
                    rc = recp.tile([64, 512], F32, tag="rc")
                    nc.vector.reciprocal(rc[:], fzs[hp][64:128, par, :])
                    nc.vector.tensor_mul(
                        outT[par * 64:(par + 1) * 64, hp, :],
                        fzs[hp][0:64, par, :], rc[:])

            # out_proj partial: y[sq, :] += outT_own^T @ wo_own. outT[:, hp]
            # partitions are exactly own-rows 128*hp..128*hp+127 of wo, so
            # the two hp chunks accumulate in PSUM. Lives in the fzp pool:
            # during the NEXT body's projection phase fz bufs are idle, so
            # the tail out_proj pipelines under it.
            for sq in range(4):
                ep = fzp.tile([128, 2, 512], F32, tag="pv")
                for half in range(2):   # matmul dst must fit one PSUM bank
                    for hp in range(2):
                        nc.tensor.matmul(
                            ep[:, half, :],
                            outT[:, hp, sq * 128:(sq + 1) * 128],
                            wo_t[:, hp, half * 512:(half + 1) * 512],
                            start=(hp == 0), stop=(hp == 1))
                yt = ysb.tile([128, E], F32, tag="y")
                nc.vector.tensor_copy(yt[:], ep.rearrange("p a b -> p (a b)"))
                nc.sync.dma_start(
                    y_d[(qc * 4 + sq) * 128:(qc * 4 + sq + 1) * 128, :],
                    yt[:])


def _get_nc():
    global _cached
    if _cached is None:
        _cached = build()
    return _cached


def _bf16(a):
    import ml_dtypes
    return np.ascontiguousarray(a, dtype=ml_dtypes.bfloat16)


def make_in_maps(x, w_qkv, w_out):
    x = np.asarray(x, dtype=np.float32)
    w_qkv = np.asarray(w_qkv, dtype=np.float32)
    w_out = np.asarray(w_out, dtype=np.float32)
    in_maps = []
    for c in range(N_CORES):
        b, r = c // 4, c % 4
        hs = r * HD                      # first qkv column of this core's heads
        in_maps.append({
            "xT": _bf16(x[b].T),
            "wq": _bf16(w_qkv[:, hs:hs + HD]),
            "wk": _bf16(w_qkv[:, E + hs:E + hs + HD]),
            "wv": _bf16(w_qkv[:, 2 * E + hs:2 * E + hs + HD]),
            "wo": _bf16(w_out[r * HD:(r + 1) * HD, :]),
        })
    return in_maps


def assemble(results):
    y = np.zeros((B, S, E), dtype=np.float32)
    for c in range(N_CORES):
        y[c // 4] += results[c]["y"]
    return y


def kernel(x, w_qkv, w_out):
    nc = _get_nc()
    res = run_bass_kernel_spmd(nc, make_in_maps(x, w_qkv, w_out),
                               list(range(N_CORES)))
    return assemble(res.results)


# revision 11
# speedup vs baseline: 3.0341x; 1.2374x over previous
"""Multi-head attention (B=2, S=2048, E=1024, H=16, D=64) on 8 Trainium2 cores.

Sharding: data-parallel over batch (2 groups of 4 cores), tensor-parallel over
heads within each group (4 heads per core, Megatron-style column-split qkv).
out_proj is sharded over its INPUT rows: each core multiplies its own 256
attention-output rows (4 heads x 64) by w_out[r*256:(r+1)*256, :] and emits a
full-width partial product y_r [2048, 1024]; the host sums the 4 partials per
batch during unsharding. That removes the device collective entirely (the
on-device AllGather measured ~0.5 ms serialized on this fabric, dominating
the kernel), at the cost of a 4x larger (8 MB) output DMA that overlaps
compute. The SPMD program contains no core-id logic at all.

All PE inputs bf16, fp32 PSUM accumulation:
  - x arrives pre-transposed from the host (x^T [1024, 2048] bf16), so the
    kernel spends zero PE cycles on transposes.
  - Projections are emitted span-wise (4 x 512 seq columns) so the matmuls
    stream behind the x^T DMA: kT for all spans first, then qT(span0), then
    v (needed progressively by the PV loop), then qT(1..3).
  - Attention per 512-row q-chunk: per k-chunk, two head-pair chains: S^T
    with two heads row-packed as K=64 pairs (concurrent on the PE), exp on
    ScalarE straight out of PSUM (scale=1/8 folded in; logits ~N(0,1) so no
    max subtraction needed), fused PV^T + softmax denominator via
    lhsT=[v_h | ones] accumulated over the 16 k-chunks.
  - PSUM: ONE shared 2-buf [128,1024] pool ("mm") serves qkv-projection
    tiles and S^T tiles (4 banks), plus a 2-buf pool (4 banks) shared by the
    PV accumulators and the out_proj partials = exactly 8 banks, so
    projections and out_proj overlap attention with no pool barrier.
  - out_proj(qc) depends only on this core's outT(qc), so it pipelines
    right behind each q-chunk's normalization.
"""

import numpy as np
from contextlib import ExitStack

import concourse.tile as tile
from concourse import bacc, mybir
from concourse.bass_utils import run_bass_kernel_spmd

B, S, E, H, D = 2, 2048, 1024, 16, 64
N_CORES = 8
HPC = 4            # heads per core
HD = HPC * D       # 256

F32 = mybir.dt.float32
BF16 = mybir.dt.bfloat16

_cached = None


def build(reps=1, profile=False):
    nc = bacc.Bacc("TRN2", target_bir_lowering=False, debug=False,
                   num_devices=N_CORES)

    xT_d = nc.dram_tensor("xT", [E, S], BF16, kind="ExternalInput").ap()
    wq_d = nc.dram_tensor("wq", [E, HD], BF16, kind="ExternalInput").ap()
    wk_d = nc.dram_tensor("wk", [E, HD], BF16, kind="ExternalInput").ap()
    wv_d = nc.dram_tensor("wv", [E, HD], BF16, kind="ExternalInput").ap()
    wo_d = nc.dram_tensor("wo", [HD, E], BF16, kind="ExternalInput").ap()
    y_d = nc.dram_tensor("y", [S, E], F32, kind="ExternalOutput").ap()

    with tile.TileContext(nc) as tc, ExitStack() as ctx:
        glob = ctx.enter_context(tc.tile_pool(name="glob", bufs=1))
        xT_t = glob.tile([128, 8, S], BF16, tag="xT")
        wq_t = glob.tile([128, 8, HD], BF16, tag="wq")
        wk_t = glob.tile([128, 8, HD], BF16, tag="wk")
        wv_t = glob.tile([128, 8, HD], BF16, tag="wv")
        wo_t = glob.tile([128, 2, E], BF16, tag="wo")
        qT_t = glob.tile([128, 2, S], BF16, tag="qT")     # q^T by head pair
        kT_t = glob.tile([128, 2, S], BF16, tag="kT")
        v_t = glob.tile([128, 16, HPC, 128], BF16, tag="v")  # [v_h | ones]
        ones_b = glob.tile([128, D], BF16, tag="ones_b")

        nc.sync.dma_start(wq_t[:], wq_d.rearrange("(c p) n -> p c n", p=128))
        nc.sync.dma_start(wk_t[:], wk_d.rearrange("(c p) n -> p c n", p=128))
        nc.sync.dma_start(wv_t[:], wv_d.rearrange("(c p) n -> p c n", p=128))
        nc.sync.dma_start(wo_t[:], wo_d.rearrange("(c p) n -> p c n", p=128))
        nc.gpsimd.memset(ones_b[:], 1.0)
        # ones half of every v_aug block (written once; v halves re-written
        # by the projection each rep)
        for sc in range(16):
            for h in range(HPC):
                nc.vector.tensor_copy(v_t[:, sc, h, 64:128], ones_b[:])

        for _rep in range(reps):
            _emit_body(nc, tc, xT_d, y_d,
                       xT_t, wq_t, wk_t, wv_t, wo_t, qT_t, kT_t, v_t)

    nc.compile()
    return nc


def _emit_body(nc, tc, xT_d, y_d,
               xT_t, wq_t, wk_t, wv_t, wo_t, qT_t, kT_t, v_t):
    xT_r = xT_d.rearrange("(c p) s -> p c s", p=128)

    with ExitStack() as body:
        mm = body.enter_context(tc.tile_pool(name="mm", bufs=2, space="PSUM"))
        fzp = body.enter_context(tc.tile_pool(name="fzp", bufs=2,
                                              space="PSUM"))
        expp = body.enter_context(tc.tile_pool(name="expp", bufs=4))
        recp = body.enter_context(tc.tile_pool(name="recp", bufs=2))
        outp = body.enter_context(tc.tile_pool(name="outp", bufs=2))
        ysb = body.enter_context(tc.tile_pool(name="ysb", bufs=3))

        # ---- x^T spans stream in; kT per span as soon as its span lands ----
        for z in range(4):
            nc.sync.dma_start(xT_t[:, :, z * 512:(z + 1) * 512],
                              xT_r[:, :, z * 512:(z + 1) * 512])

        def proj_T(w_t, dst, z):
            # dst[:, mc, span] = (w col-block mc)^T @ x^T span
            for mc in range(2):
                pp = mm.tile([128, 1024], F32, tag="st")
                for ec in range(8):
                    nc.tensor.matmul(
                        pp[:, 0:512],
                        w_t[:, ec, mc * 128:(mc + 1) * 128],
                        xT_t[:, ec, z * 512:(z + 1) * 512],
                        start=(ec == 0), stop=(ec == 7))
                nc.vector.tensor_copy(
                    dst[:, mc, z * 512:(z + 1) * 512], pp[:, 0:512])

        def proj_v(z):
            for i in range(4):
                sc = z * 4 + i
                pp = mm.tile([128, 1024], F32, tag="st")
                for ec in range(8):
                    nc.tensor.matmul(
                        pp[:, 0:HD],
                        xT_t[:, ec, sc * 128:(sc + 1) * 128],
                        wv_t[:, ec, :],
                        start=(ec == 0), stop=(ec == 7))
                nc.vector.tensor_copy(
                    v_t[:, sc, :, 0:64],
                    pp[:, 0:HD].rearrange("p (h d) -> p h d", h=HPC))

        for z in range(4):
            proj_T(wk_t, kT_t, z)
        proj_T(wq_t, qT_t, 0)
        for z in range(4):
            proj_v(z)
        for z in range(1, 4):
            proj_T(wq_t, qT_t, z)

        # ---- attention + row-partial out_proj, per 512-row q-chunk ----
        for qc in range(4):
            fz0 = fzp.tile([128, 2, 512], F32, tag="pv")
            fz1 = fzp.tile([128, 2, 512], F32, tag="pv")
            fzs = [fz0, fz1]
            for kc in range(16):
                sts = []
                for hp in range(2):
                    st = mm.tile([128, 1024], F32, tag="st")
                    for par in range(2):   # row-packed K=64 pair
                        lo, hi = par * 64, (par + 1) * 64
                        nc.tensor.matmul(
                            st[:, par * 512:(par + 1) * 512],
                            kT_t[lo:hi, hp, kc * 128:(kc + 1) * 128],
                            qT_t[lo:hi, hp, qc * 512:(qc + 1) * 512],
                            start=True, stop=True)
                    sts.append(st)
                exs = []
                for hp in range(2):
                    ex = expp.tile([128, 1024], BF16, tag="ex")
                    nc.scalar.activation(
                        ex[:], sts[hp][:],
                        mybir.ActivationFunctionType.Exp, scale=0.125)
                    exs.append(ex)
                for hp in range(2):
                    for par in range(2):
                        h = 2 * hp + par
                        nc.tensor.matmul(
                            fzs[hp][:, par, :],
                            v_t[:, kc, h, :],
                            exs[hp][:, par * 512:(par + 1) * 512],
                            start=(kc == 0), stop=(kc == 15))
            outT = outp.tile([128, 2, 512], BF16, tag="outT")
            for hp in range(2):
                for par in range(2):
                    rc = recp.tile([64, 512], F32, tag="rc")
                    nc.vector.reciprocal(rc[:], fzs[hp][64:128, par, :])
                    nc.vector.tensor_mul(
                        outT[par * 64:(par + 1) * 64, hp, :],
                        fzs[hp][0:64, par, :], rc[:])

            # out_proj partial: y[sq, :] += outT_own^T @ wo_own. outT[:, hp]
            # partitions are exactly own-rows 128*hp..128*hp+127 of wo, so
            # the two hp chunks accumulate in PSUM. Lives in the fzp pool:
            # during the NEXT body's projection phase fz bufs are idle, so
            # the tail out_proj pipelines under it.
            for sq in range(4):
                ep = fzp.tile([128, 2, 512], F32, tag="pv")
                for half in range(2):   # matmul dst must fit one PSUM bank
                    for hp in range(2):
                        nc.tensor.matmul(
                            ep[:, half, :],
                            outT[:, hp, sq * 128:(sq + 1) * 128],
                            wo_t[:, hp, half * 512:(half + 1) * 512],
                            start=(hp == 0), stop=(hp == 1))
                yt = ysb.tile([128, E], F32, tag="y")
                nc.vector.tensor_copy(yt[:], ep.rearrange("p a b -> p (a b)"))
                nc.sync.dma_start(
                    y_d[(qc * 4 + sq) * 128:(qc * 4 + sq + 1) * 128, :],
                    yt[:])


def _get_nc():
    global _cached
    if _cached is None:
        _cached = build()
    return _cached


def _bf16(a):
    import ml_dtypes
    return np.ascontiguousarray(a, dtype=ml_dtypes.bfloat16)


def make_in_maps(x, w_qkv, w_out):
    x = np.asarray(x, dtype=np.float32)
    w_qkv = np.asarray(w_qkv, dtype=np.float32)
    w_out = np.asarray(w_out, dtype=np.float32)
    in_maps = []
    for c in range(N_CORES):
        b, r = c // 4, c % 4
        hs = r * HD                      # first qkv column of this core's heads
        in_maps.append({
            "xT": _bf16(x[b].T),
            "wq": _bf16(w_qkv[:, hs:hs + HD]),
            "wk": _bf16(w_qkv[:, E + hs:E + hs + HD]),
            "wv": _bf16(w_qkv[:, 2 * E + hs:2 * E + hs + HD]),
            "wo": _bf16(w_out[r * HD:(r + 1) * HD, :]),
        })
    return in_maps


def assemble(results):
    y = np.zeros((B, S, E), dtype=np.float32)
    for c in range(N_CORES):
        y[c // 4] += results[c]["y"]
    return y


def kernel(x, w_qkv, w_out):
    nc = _get_nc()
    res = run_bass_kernel_spmd(nc, make_in_maps(x, w_qkv, w_out),
                               list(range(N_CORES)))
    return assemble(res.results)
